# revision 27
# baseline (speedup 1.0000x reference)
"""Trainium2 Bass kernel for nn_ConvOverTimeLayer.

Computes out[b,0,c,h,w] = sum_t x[b,t,c,h,w] * W[c,t] + bias[c]
(1024 independent per-map 1x1 convs over a 10-channel time axis).

Strategy:
  - Data-parallel over batch: 16 batches -> 8 cores x 2 batches.
  - Per core, per 128-channel block: accumulate the t-contraction on the
    TensorEngine as 10 PSUM-accumulated matmuls with diagonal weight
    matrices diag(W[cblk, t]) (K = c = 128, moving N = 2*196 = 392),
    since x's natural [c, hw] layout puts channels on partitions.
  - Diag matrices are built on-chip: eye * W[:, t] (per-partition scalar).
  - Bias is fused into the PSUM->SBUF evacuation (tensor_scalar add).
"""

import sys

import numpy as np

for _p in ("/opt/trn_rl_repo",):
    if _p not in sys.path:
        sys.path.insert(0, _p)

import concourse.bass as bass
import concourse.bacc as bacc
import concourse.mybir as mybir
from concourse.bass_utils import run_bass_kernel_spmd
from concourse.tile import TileContext

B, T, C, H, W_DIM = 16, 10, 1024, 14, 14
HW = H * W_DIM  # 196
NCORES = 8
B_LOC = B // NCORES  # 2 batches per core
P = 128  # channels per block = SBUF partitions
NBLK = C // P  # 8 channel blocks per core
F32 = mybir.dt.float32
F16 = mybir.dt.float16

_NC = None


# Packed constants tensor layout (single DMA => single semaphore; the
# DVE TensorScalarPtr encoding only has one sync-wait slot, so its inputs
# must all arrive via one DMA): [128, NBLK*T (W) | NBLK (bias) | P (eye)]
WBE_W0 = 0  # W block: [128, NBLK, T]
WBE_B0 = NBLK * T  # bias block: [128, NBLK]
WBE_E0 = WBE_B0 + NBLK  # eye block: [128, P]
WBE_COLS = WBE_E0 + P


def _build_nc():
    nc = bacc.Bacc()
    x = nc.declare_dram_parameter("x", [B_LOC, T, C, H, W_DIM], F16, isOutput=False)
    wbe = nc.declare_dram_parameter("wbe", [P, WBE_COLS], F32, isOutput=False)
    out = nc.declare_dram_parameter("out", [B_LOC, 1, C, H, W_DIM], F32, isOutput=True)

    # Channel-block views with c on the partition axis. t outermost in the
    # tile's free dims so the load can be split into t-halves across the two
    # HWDGE engines (sync + scalar) for parallel queue processing.
    x_v = x.rearrange("b t (n p) h w -> n p b t (h w)", p=P)  # [NBLK,128,2,10,196]
    out_v = out.rearrange("b o (n p) h w -> n p b (o h w)", p=P)  # [NBLK,128,2,196]

    with TileContext(nc) as tc:
        with (
            tc.tile_pool(name="const", bufs=1) as cpool,
            tc.tile_pool(name="xin", bufs=7) as xpool,
            tc.tile_pool(name="diag", bufs=1) as dpool,
            tc.tile_pool(name="psum", bufs=8, space="PSUM") as ppool,
            tc.tile_pool(name="outp", bufs=NBLK) as opool,
        ):
            wbe_tile = cpool.tile([P, WBE_COLS], F32, tag="wbe")
            nc.sync.dma_start(out=wbe_tile[:], in_=wbe[:])
            w_view = wbe_tile[:, WBE_W0:WBE_B0].rearrange(
                "p (n t) -> p n t", t=T
            )  # [128, NBLK, T]
            eye_view = wbe_tile[:, WBE_E0:WBE_COLS]  # [128, 128]

            # PE pstate warmup: the PE clock ramps to peak only after ~3-4us
            # of sustained activity. Run throwaway matmuls on a memset tile
            # during the prologue/first-load dead time so the real matmuls
            # start at full clock. Uses the same psum slot pool (tag reuse).
            warm = cpool.tile([P, P], F16, tag="warm")
            nc.vector.memset(warm[:], 0.0)
            wacc = ppool.tile([P, B_LOC, HW], F32, tag="acc")
            for i in range(40):
                nc.tensor.matmul(
                    wacc[:, 0, :P],
                    warm[:],
                    warm[:],
                    start=True,
                    stop=True,
                )

            # diag(W[cblk, t]) for every (block, t): eye * per-partition scalar.
            diags = {}
            for n in range(NBLK):
                for t in range(T):
                    d = dpool.tile([P, P], F16, tag=f"diag_{n}_{t}")
                    nc.vector.tensor_scalar(
                        out=d[:],
                        in0=eye_view,
                        scalar1=w_view[:, n, t : t + 1],
                        scalar2=None,
                        op0=mybir.AluOpType.mult,
                    )
                    diags[(n, t)] = d

            TH = T // 2
            outs = []
            for n in range(NBLK):
                # t-major tile: each matmul's moving operand xt[:, t] is one
                # contiguous 392-element run per partition. 4 chunks per block
                # (batch x t-half) spread over both HWDGE engines: t<TH
                # matmuls start after the first pair, and the small chunks
                # keep both DGE queues at their peak rate.
                xt = xpool.tile([P, T, B_LOC, HW], F16, tag="x")
                if n == 0:
                    nc.sync.dma_start(out=xt[:, :TH, 0, :], in_=x_v[n, :, 0, :TH, :])
                    nc.scalar.dma_start(out=xt[:, :TH, 1, :], in_=x_v[n, :, 1, :TH, :])
                    nc.sync.dma_start(out=xt[:, TH:, 0, :], in_=x_v[n, :, 0, TH:, :])
                    nc.scalar.dma_start(out=xt[:, TH:, 1, :], in_=x_v[n, :, 1, TH:, :])
                else:
                    nc.sync.dma_start(out=xt[:, :, 0, :], in_=x_v[n, :, 0, :, :])
                    nc.scalar.dma_start(out=xt[:, :, 1, :], in_=x_v[n, :, 1, :, :])
                acc = ppool.tile([P, B_LOC, HW], F32, tag="acc")
                for t in range(T):
                    # f16 matmul: 1 cyc/row + fast weight load; accumulation
                    # stays fp32 in PSUM.
                    nc.tensor.matmul(
                        acc[:],
                        diags[(n, t)][:],
                        xt[:, t, :, :],
                        start=(t == 0),
                        stop=(t == T - 1),
                    )
                # Per-block output tile (bufs=NBLK: no slot reuse, so evacs
                # never wait on out-DMAs).
                ot = opool.tile([P, B_LOC, HW], F32, tag=f"o_{n}")
                nc.vector.tensor_scalar(
                    out=ot[:],
                    in0=acc[:],
                    scalar1=wbe_tile[:, WBE_B0 + n : WBE_B0 + n + 1],
                    scalar2=None,
                    op0=mybir.AluOpType.add,
                )
                outs.append((n, ot))

            # All out-DMAs are queued AFTER every x-load: an out-DMA waits on
            # its evac, and placing one ahead of a later load in the same
            # FIFO queue would stall that load (head-of-line blocking).
            for m, mt in outs:
                (nc.scalar if m % 2 == 0 else nc.sync).dma_start(
                    out=out_v[m], in_=mt[:]
                )
    nc.compile()
    return nc


def _get_nc():
    global _NC
    if _NC is None:
        _NC = _build_nc()
    return _NC


def _run(in_maps, **kwargs):
    return run_bass_kernel_spmd(_get_nc(), in_maps, list(range(NCORES)), **kwargs)


def _make_in_maps(input, W, b):
    x = np.asarray(input, dtype=np.float32).astype(np.float16)
    W = np.asarray(W, dtype=np.float32)
    b = np.asarray(b, dtype=np.float32)
    wbe = np.empty((P, WBE_COLS), dtype=np.float32)
    # W[c, t] with c = n*P + p  ->  wbe[p, n*T + t]
    wbe[:, WBE_W0:WBE_B0] = W.reshape(NBLK, P, T).transpose(1, 0, 2).reshape(P, -1)
    wbe[:, WBE_B0:WBE_E0] = b.reshape(NBLK, P).T
    wbe[:, WBE_E0:WBE_COLS] = np.eye(P, dtype=np.float32)
    return [
        {
            "x": x[i * B_LOC : (i + 1) * B_LOC],
            "wbe": wbe,
        }
        for i in range(NCORES)
    ]


def kernel(input, W, b):
    in_maps = _make_in_maps(input, W, b)
    res = _run(in_maps).results
    return np.concatenate([r["out"] for r in res], axis=0)


# revision 28
# speedup vs baseline: 1.0204x; 1.0204x over previous
"""Trainium2 Bass kernel for nn_ConvOverTimeLayer.

Computes out[b,0,c,h,w] = sum_t x[b,t,c,h,w] * W[c,t] + bias[c]
(1024 independent per-map 1x1 convs over a 10-channel time axis).

Strategy:
  - Data-parallel over batch: 16 batches -> 8 cores x 2 batches.
  - Per core, per 128-channel block: accumulate the t-contraction on the
    TensorEngine as 10 PSUM-accumulated matmuls with diagonal weight
    matrices diag(W[cblk, t]) (K = c = 128, moving N = 2*196 = 392),
    since x's natural [c, hw] layout puts channels on partitions.
  - Diag matrices are built on-chip: eye * W[:, t] (per-partition scalar).
  - Bias is fused into the PSUM->SBUF evacuation (tensor_scalar add).
"""

import sys

import numpy as np

for _p in ("/opt/trn_rl_repo",):
    if _p not in sys.path:
        sys.path.insert(0, _p)

import concourse.bass as bass
import concourse.bacc as bacc
import concourse.mybir as mybir
from concourse.bass_utils import run_bass_kernel_spmd
from concourse.tile import TileContext

B, T, C, H, W_DIM = 16, 10, 1024, 14, 14
HW = H * W_DIM  # 196
NCORES = 8
B_LOC = B // NCORES  # 2 batches per core
P = 128  # channels per block = SBUF partitions
NBLK = C // P  # 8 channel blocks per core
F32 = mybir.dt.float32
F16 = mybir.dt.float16

_NC = None


# Packed constants tensor layout (single DMA => single semaphore; the
# DVE TensorScalarPtr encoding only has one sync-wait slot, so its inputs
# must all arrive via one DMA): [128, NBLK*T (W) | NBLK (bias) | P (eye)]
WBE_W0 = 0  # W block: [128, NBLK, T]
WBE_B0 = NBLK * T  # bias block: [128, NBLK]
WBE_E0 = WBE_B0 + NBLK  # eye block: [128, P]
WBE_COLS = WBE_E0 + P


def _build_nc():
    nc = bacc.Bacc()
    x = nc.declare_dram_parameter("x", [B_LOC, T, C, H, W_DIM], F16, isOutput=False)
    wbe = nc.declare_dram_parameter("wbe", [P, WBE_COLS], F32, isOutput=False)
    out = nc.declare_dram_parameter("out", [B_LOC, 1, C, H, W_DIM], F32, isOutput=True)

    # Channel-block views with c on the partition axis. t outermost in the
    # tile's free dims so the load can be split into t-halves across the two
    # HWDGE engines (sync + scalar) for parallel queue processing.
    x_v = x.rearrange("b t (n p) h w -> n p b t (h w)", p=P)  # [NBLK,128,2,10,196]
    out_v = out.rearrange("b o (n p) h w -> n p b (o h w)", p=P)  # [NBLK,128,2,196]

    with TileContext(nc) as tc:
        with (
            tc.tile_pool(name="const", bufs=1) as cpool,
            tc.tile_pool(name="xin", bufs=7) as xpool,
            tc.tile_pool(name="diag", bufs=1) as dpool,
            tc.tile_pool(name="psum", bufs=8, space="PSUM") as ppool,
            tc.tile_pool(name="outp", bufs=NBLK) as opool,
        ):
            wbe_tile = cpool.tile([P, WBE_COLS], F32, tag="wbe")
            nc.sync.dma_start(out=wbe_tile[:], in_=wbe[:])
            w_view = wbe_tile[:, WBE_W0:WBE_B0].rearrange(
                "p (n t) -> p n t", t=T
            )  # [128, NBLK, T]
            eye_view = wbe_tile[:, WBE_E0:WBE_COLS]  # [128, 128]

            # PE pstate warmup: the PE clock ramps to peak only after ~3-4us
            # of sustained activity. Run throwaway matmuls on a memset tile
            # during the prologue/first-load dead time so the real matmuls
            # start at full clock. Uses the same psum slot pool (tag reuse).
            warm = cpool.tile([P, P], F16, tag="warm")
            nc.vector.memset(warm[:], 0.0)
            wacc = ppool.tile([P, B_LOC, HW], F32, tag="acc")
            for i in range(40):
                nc.tensor.matmul(
                    wacc[:, 0, :P],
                    warm[:],
                    warm[:],
                    start=True,
                    stop=True,
                )

            # diag(W[cblk, t]) for every (block, t): eye * per-partition scalar.
            diags = {}
            for n in range(NBLK):
                for t in range(T):
                    d = dpool.tile([P, P], F16, tag=f"diag_{n}_{t}")
                    nc.vector.tensor_scalar(
                        out=d[:],
                        in0=eye_view,
                        scalar1=w_view[:, n, t : t + 1],
                        scalar2=None,
                        op0=mybir.AluOpType.mult,
                    )
                    diags[(n, t)] = d

            TH = T // 2
            outs = []
            for n in range(NBLK):
                # t-major tile: each matmul's moving operand xt[:, t] is one
                # contiguous 392-element run per partition. 4 chunks per block
                # (batch x t-half) spread over both HWDGE engines: t<TH
                # matmuls start after the first pair, and the small chunks
                # keep both DGE queues at their peak rate.
                xt = xpool.tile([P, T, B_LOC, HW], F16, tag="x")
                for lo, hi in ((0, 2), (2, TH), (TH, T)):
                    nc.sync.dma_start(
                        out=xt[:, lo:hi, 0, :], in_=x_v[n, :, 0, lo:hi, :]
                    )
                    nc.scalar.dma_start(
                        out=xt[:, lo:hi, 1, :], in_=x_v[n, :, 1, lo:hi, :]
                    )
                acc = ppool.tile([P, B_LOC, HW], F32, tag="acc")
                for t in range(T):
                    # f16 matmul: 1 cyc/row + fast weight load; accumulation
                    # stays fp32 in PSUM.
                    nc.tensor.matmul(
                        acc[:],
                        diags[(n, t)][:],
                        xt[:, t, :, :],
                        start=(t == 0),
                        stop=(t == T - 1),
                    )
                # Per-block output tile (bufs=NBLK: no slot reuse, so evacs
                # never wait on out-DMAs).
                ot = opool.tile([P, B_LOC, HW], F32, tag=f"o_{n}")
                nc.vector.tensor_scalar(
                    out=ot[:],
                    in0=acc[:],
                    scalar1=wbe_tile[:, WBE_B0 + n : WBE_B0 + n + 1],
                    scalar2=None,
                    op0=mybir.AluOpType.add,
                )
                outs.append((n, ot))

            # All out-DMAs are queued AFTER every x-load: an out-DMA waits on
            # its evac, and placing one ahead of a later load in the same
            # FIFO queue would stall that load (head-of-line blocking).
            for m, mt in outs:
                (nc.scalar if m % 2 == 0 else nc.sync).dma_start(
                    out=out_v[m], in_=mt[:]
                )
    nc.compile()
    return nc


def _get_nc():
    global _NC
    if _NC is None:
        _NC = _build_nc()
    return _NC


def _run(in_maps, **kwargs):
    return run_bass_kernel_spmd(_get_nc(), in_maps, list(range(NCORES)), **kwargs)


def _make_in_maps(input, W, b):
    x = np.asarray(input, dtype=np.float32).astype(np.float16)
    W = np.asarray(W, dtype=np.float32)
    b = np.asarray(b, dtype=np.float32)
    wbe = np.empty((P, WBE_COLS), dtype=np.float32)
    # W[c, t] with c = n*P + p  ->  wbe[p, n*T + t]
    wbe[:, WBE_W0:WBE_B0] = W.reshape(NBLK, P, T).transpose(1, 0, 2).reshape(P, -1)
    wbe[:, WBE_B0:WBE_E0] = b.reshape(NBLK, P).T
    wbe[:, WBE_E0:WBE_COLS] = np.eye(P, dtype=np.float32)
    return [
        {
            "x": x[i * B_LOC : (i + 1) * B_LOC],
            "wbe": wbe,
        }
        for i in range(NCORES)
    ]


def kernel(input, W, b):
    in_maps = _make_in_maps(input, W, b)
    res = _run(in_maps).results
    return np.concatenate([r["out"] for r in res], axis=0)


# revision 30
# speedup vs baseline: 1.0436x; 1.0227x over previous
"""Trainium2 Bass kernel for nn_ConvOverTimeLayer.

Computes out[b,0,c,h,w] = sum_t x[b,t,c,h,w] * W[c,t] + bias[c]
(1024 independent per-map 1x1 convs over a 10-channel time axis).

Strategy:
  - Data-parallel over batch: 16 batches -> 8 cores x 2 batches.
  - Per core, per 128-channel block: accumulate the t-contraction on the
    TensorEngine as 10 PSUM-accumulated matmuls with diagonal weight
    matrices diag(W[cblk, t]) (K = c = 128, moving N = 2*196 = 392),
    since x's natural [c, hw] layout puts channels on partitions.
  - Diag matrices are built on-chip: eye * W[:, t] (per-partition scalar).
  - Bias is fused into the PSUM->SBUF evacuation (tensor_scalar add).
"""

import sys

import numpy as np

for _p in ("/opt/trn_rl_repo",):
    if _p not in sys.path:
        sys.path.insert(0, _p)

import concourse.bass as bass
import concourse.bacc as bacc
import concourse.mybir as mybir
from concourse.bass_utils import run_bass_kernel_spmd
from concourse.tile import TileContext

B, T, C, H, W_DIM = 16, 10, 1024, 14, 14
HW = H * W_DIM  # 196
NCORES = 8
B_LOC = B // NCORES  # 2 batches per core
P = 128  # channels per block = SBUF partitions
NBLK = C // P  # 8 channel blocks per core
F32 = mybir.dt.float32
F16 = mybir.dt.float16

_NC = None


# Packed constants tensor layout (single DMA => single semaphore; the
# DVE TensorScalarPtr encoding only has one sync-wait slot, so its inputs
# must all arrive via one DMA): [128, NBLK*T (W) | NBLK (bias) | P (eye)]
WBE_W0 = 0  # W block: [128, NBLK, T]
WBE_B0 = NBLK * T  # bias block: [128, NBLK]
WBE_E0 = WBE_B0 + NBLK  # eye block: [128, P]
WBE_COLS = WBE_E0 + P


def _build_nc():
    nc = bacc.Bacc()
    x = nc.declare_dram_parameter("x", [B_LOC, T, C, H, W_DIM], F16, isOutput=False)
    wbe = nc.declare_dram_parameter("wbe", [P, WBE_COLS], F32, isOutput=False)
    out = nc.declare_dram_parameter("out", [B_LOC, 1, C, H, W_DIM], F32, isOutput=True)

    # Channel-block views with c on the partition axis. t outermost in the
    # tile's free dims so the load can be split into t-halves across the two
    # HWDGE engines (sync + scalar) for parallel queue processing.
    x_v = x.rearrange("b t (n p) h w -> n p b t (h w)", p=P)  # [NBLK,128,2,10,196]
    out_v = out.rearrange("b o (n p) h w -> n p b (o h w)", p=P)  # [NBLK,128,2,196]

    with TileContext(nc) as tc:
        with (
            tc.tile_pool(name="const", bufs=1) as cpool,
            tc.tile_pool(name="xin", bufs=7) as xpool,
            tc.tile_pool(name="diag", bufs=1) as dpool,
            tc.tile_pool(name="psum", bufs=8, space="PSUM") as ppool,
            tc.tile_pool(name="outp", bufs=NBLK) as opool,
        ):
            wbe_tile = cpool.tile([P, WBE_COLS], F32, tag="wbe")
            nc.sync.dma_start(out=wbe_tile[:], in_=wbe[:])
            w_view = wbe_tile[:, WBE_W0:WBE_B0].rearrange(
                "p (n t) -> p n t", t=T
            )  # [128, NBLK, T]
            eye_view = wbe_tile[:, WBE_E0:WBE_COLS]  # [128, 128]

            # diag(W[cblk, t]) for every (block, t): eye * per-partition scalar.
            diags = {}
            for n in range(NBLK):
                for t in range(T):
                    d = dpool.tile([P, P], F16, tag=f"diag_{n}_{t}")
                    nc.vector.tensor_scalar(
                        out=d[:],
                        in0=eye_view,
                        scalar1=w_view[:, n, t : t + 1],
                        scalar2=None,
                        op0=mybir.AluOpType.mult,
                    )
                    diags[(n, t)] = d

            TH = T // 2
            outs = []
            for n in range(NBLK):
                # t-major tile: each matmul's moving operand xt[:, t] is one
                # contiguous 392-element run per partition. 4 chunks per block
                # (batch x t-half) spread over both HWDGE engines: t<TH
                # matmuls start after the first pair, and the small chunks
                # keep both DGE queues at their peak rate.
                xt = xpool.tile([P, T, B_LOC, HW], F16, tag="x")
                nc.sync.dma_start(out=xt[:, :TH, 0, :], in_=x_v[n, :, 0, :TH, :])
                nc.scalar.dma_start(out=xt[:, :TH, 1, :], in_=x_v[n, :, 1, :TH, :])
                nc.sync.dma_start(out=xt[:, TH:, 0, :], in_=x_v[n, :, 0, TH:, :])
                nc.scalar.dma_start(out=xt[:, TH:, 1, :], in_=x_v[n, :, 1, TH:, :])
                acc = ppool.tile([P, B_LOC, HW], F32, tag="acc")
                for t in range(T):
                    # f16 matmul: 1 cyc/row + fast weight load; accumulation
                    # stays fp32 in PSUM.
                    nc.tensor.matmul(
                        acc[:],
                        diags[(n, t)][:],
                        xt[:, t, :, :],
                        start=(t == 0),
                        stop=(t == T - 1),
                    )
                # Per-block output tile (bufs=NBLK: no slot reuse, so evacs
                # never wait on out-DMAs).
                ot = opool.tile([P, B_LOC, HW], F32, tag=f"o_{n}")
                nc.vector.tensor_scalar(
                    out=ot[:],
                    in0=acc[:],
                    scalar1=wbe_tile[:, WBE_B0 + n : WBE_B0 + n + 1],
                    scalar2=None,
                    op0=mybir.AluOpType.add,
                )
                outs.append((n, ot))

            # All out-DMAs are queued AFTER every x-load: an out-DMA waits on
            # its evac, and placing one ahead of a later load in the same
            # FIFO queue would stall that load (head-of-line blocking).
            for m, mt in outs:
                (nc.scalar if m % 2 == 0 else nc.sync).dma_start(
                    out=out_v[m], in_=mt[:]
                )
    nc.compile()
    return nc


def _get_nc():
    global _NC
    if _NC is None:
        _NC = _build_nc()
    return _NC


def _run(in_maps, **kwargs):
    return run_bass_kernel_spmd(_get_nc(), in_maps, list(range(NCORES)), **kwargs)


def _make_in_maps(input, W, b):
    x = np.asarray(input, dtype=np.float32).astype(np.float16)
    W = np.asarray(W, dtype=np.float32)
    b = np.asarray(b, dtype=np.float32)
    wbe = np.empty((P, WBE_COLS), dtype=np.float32)
    # W[c, t] with c = n*P + p  ->  wbe[p, n*T + t]
    wbe[:, WBE_W0:WBE_B0] = W.reshape(NBLK, P, T).transpose(1, 0, 2).reshape(P, -1)
    wbe[:, WBE_B0:WBE_E0] = b.reshape(NBLK, P).T
    wbe[:, WBE_E0:WBE_COLS] = np.eye(P, dtype=np.float32)
    return [
        {
            "x": x[i * B_LOC : (i + 1) * B_LOC],
            "wbe": wbe,
        }
        for i in range(NCORES)
    ]


def kernel(input, W, b):
    in_maps = _make_in_maps(input, W, b)
    res = _run(in_maps).results
    return np.concatenate([r["out"] for r in res], axis=0)
